# revision 25
# baseline (speedup 1.0000x reference)
"""AM-softmax mixup loss (nn_MixupTrainLoss) on 8 TRN2 NeuronCores.

Strategy (classification / tensor parallel over the class dim, per the
sharding hint):
  - Host: L2-normalize x [512,256] and W [100000,256] rows, transpose W,
    cast to fp16 (measured end-to-end loss error ~1e-6).
  - Shard W.T column-wise: core i gets classes [12500*i, 12500*(i+1)).
    Slab layout per core: [256 gathered target cols | 12500 real | 44 pad].
  - Each core: cos = x_norm @ wn.T slab via fp16 matmuls (PE, fp32 PSUM),
    exp(S*cos) + row-sum fused on ScalarE over 2048-wide PSUM groups
    (accum_out) -> local sum_c exp(S*cos).
  - The <=4 margin-modified entries per row only perturb the softmax sum
    at <=4 columns; the gathered W columns reproduce the slab cos values
    bit-identically, so the host applies the exact margin/overwrite
    corrections analytically, merges the 8 partial sums, and finishes the
    (tiny) cross-entropy reduction in float64.
"""
import os

import numpy as np

import concourse.bacc as bacc
import concourse.bass as bass
import concourse.tile as tile
from concourse import mybir
from concourse.bass_utils import run_bass_kernel_spmd

F32 = mybir.dt.float32
F16 = mybir.dt.float16

B = 512          # batch
D = 256          # feature dim
C = 100000       # num classes
S = 30.0         # AM-softmax scale
MARGIN = 0.2     # AM-softmax margin
EPS = 1e-12
NCORES = 8
CLOC = C // NCORES          # 12500 real classes per core
CHUNK = 512                 # matmul moving free dim (one fp32 PSUM bank)
NCHUNK = 25                 # 25 chunks of 512 = 12800 slab cols per core
NG = 4 * B // NCORES        # 256 gathered target cols (slab cols [0, 256))
REAL_END = NG + CLOC        # 12756; slab cols [12756, 12800) are zero pad
NM = B // 128               # 4 row tiles of 128
GSIZES = [1, 2, 4, 4, 4, 4, 4, 2]  # chunk groups (4 banks max), small first for fast pipeline fill

_CACHE: dict = {}


def _build():
    if "nc" in _CACHE:
        return _CACHE["nc"]
    nc = bacc.Bacc("TRN2", target_bir_lowering=False, debug=False)
    wT = nc.dram_tensor("wT", [D, NCHUNK * CHUNK], F16, kind="ExternalInput")
    xT = nc.dram_tensor("xT", [D, B], F16, kind="ExternalInput")
    sumexp = nc.dram_tensor("sumexp", [128, NM * 16], F32, kind="ExternalOutput")
    cosg = nc.dram_tensor("cosg", [NM, 128, NG], F32, kind="ExternalOutput")

    def wT_g(c0, w):
        return wT[:, c0:c0 + w].rearrange("(kh p) n -> p kh n", p=128)

    xT_t = xT.rearrange("(kh p) b -> p kh b", p=128)

    with tile.TileContext(nc) as tc:
        with (
            tc.tile_pool(name="xpool", bufs=1) as xpool,
            tc.tile_pool(name="wpool", bufs=1) as wpool,
            tc.tile_pool(name="epool", bufs=2) as epool,
            tc.tile_pool(name="apool", bufs=1) as apool,
            tc.tile_pool(name="opool", bufs=1) as opool,
            tc.tile_pool(name="gpool", bufs=4) as gpool,
            tc.tile_pool(name="ps", bufs=2, space="PSUM") as pspool,
        ):
            t_x = xpool.tile([128, 2, B], F16)
            nc.sync.dma_start(t_x[:], xT_t)

            wgs = []
            ch0 = 0
            for g, gs in enumerate(GSIZES):
                t_w = wpool.tile([128, 2, gs * CHUNK], F16, tag=f"w{g}", name=f"w{g}")
                eng = nc.gpsimd if g == 0 else nc.sync
                eng.dma_start(
                    t_w[:], wT_g(ch0 * CHUNK, gs * CHUNK))
                wgs.append(t_w)
                ch0 += gs

            acc_all = apool.tile([128, NM * 16], F32, name="acc_all")
            nc.vector.memset(acc_all[:], 0.0)

            # tiny warm-up exp so the ACT table load happens during the
            # initial DMA wait instead of on the critical path
            t_wu = opool.tile([128, 1], F32, name="warmup")
            nc.vector.memset(t_wu[:], 0.0)
            nc.scalar.activation(
                t_wu[:], t_wu[:], mybir.ActivationFunctionType.Exp,
            )
            # dummy matmuls on zeros during the initial DMA wait: keeps the
            # PE busy >3.4us so the HAM clock gate opens before real work
            t_z = opool.tile([128, CHUNK], F16, name="warmz")
            nc.vector.memset(t_z[:], 0.0)
            ps_wu = pspool.tile([128, 4 * CHUNK], F32, tag="ps")
            for r in range(6):
                nc.tensor.matmul(
                    ps_wu[:, 0:CHUNK], t_z[:, 0:128], t_z[:],
                    start=True, stop=True,
                )

            ch0 = 0
            for g, gs in enumerate(GSIZES):
                W = gs * CHUNK
                for m in range(NM):
                    ps = pspool.tile([128, 4 * CHUNK], F32, tag="ps")
                    for j in range(gs):
                        for kh in range(2):
                            nc.tensor.matmul(
                                ps[:, j * CHUNK:(j + 1) * CHUNK],
                                t_x[:, kh, bass.ts(m, 128)],
                                wgs[g][:, kh, j * CHUNK:(j + 1) * CHUNK],
                                start=(kh == 0),
                                stop=(kh == 1),
                            )
                    # exp over the real-class region of this group
                    lo = NG if g == 0 else 0
                    hi = min(W, REAL_END - ch0 * CHUNK)
                    t_exp = epool.tile([128, 4 * CHUNK], F32, tag="exp")
                    nc.scalar.activation(
                        t_exp[:, lo:hi],
                        ps[:, lo:hi],
                        mybir.ActivationFunctionType.Exp,
                        scale=S,
                        accum_out=acc_all[:, m * 16 + g:m * 16 + g + 1],
                    )
                    if g == 0:
                        t_g = gpool.tile([128, NG], F32, tag="g")
                        nc.vector.tensor_copy(t_g[:], ps[:, 0:NG])
                        nc.gpsimd.dma_start(cosg[m], t_g[:])
                ch0 += gs

            nc.sync.dma_start(sumexp[:], acc_all[:])

    nc.finalize()
    _CACHE["nc"] = nc
    return nc


def kernel(inputs, weight, lam, targets1, pre1, targets2, pre2):
    inputs = np.asarray(inputs, dtype=np.float32)
    weight = np.asarray(weight, dtype=np.float32)
    lam = float(np.asarray(lam))
    tgts = [np.asarray(t).astype(np.int64) for t in (targets1, pre1, targets2, pre2)]

    # ---- host prep: normalize (float64 for accuracy), cast to fp16 ----
    x = inputs[:, :, 0].astype(np.float64)
    xn = (x / np.maximum(np.sqrt((x * x).sum(1, keepdims=True)), EPS)).astype(np.float16)
    w = weight.astype(np.float64)
    wn = (w / np.maximum(np.sqrt((w * w).sum(1, keepdims=True)), EPS)).astype(np.float16)

    xT = np.ascontiguousarray(xn.T)                      # [256, 512] fp16

    # pair p = k*B + b  (k in 0..3 over [targets1, pre1, targets2, pre2])
    cols = np.concatenate([t for t in tgts])             # [2048]

    in_maps = []
    for i in range(NCORES):
        slab = np.zeros((D, NCHUNK * CHUNK), dtype=np.float16)
        slab[:, :NG] = wn[cols[i * NG:(i + 1) * NG]].T   # gathered target cols
        slab[:, NG:REAL_END] = wn[i * CLOC:(i + 1) * CLOC].T
        in_maps.append({"wT": slab, "xT": xT})

    nc = _build()
    trace = bool(int(os.environ.get("KERNEL_TRACE", "0")))
    res = run_bass_kernel_spmd(nc, in_maps, core_ids=list(range(NCORES)), trace=trace)
    kernel.last_results = res

    # ---- host combine (float64, tiny) ----
    sumexp = np.zeros(B, dtype=np.float64)
    cosg = np.empty(4 * B, dtype=np.float32)             # raw cos at pair cols
    for i, out in enumerate(res.results):
        se = out["sumexp"].astype(np.float64)            # [128, NM*16]
        se = se.reshape(128, NM, 16).sum(2)              # [128, NM]
        sumexp += se.T.reshape(B)                        # b = m*128 + p
        cg = out["cosg"]                                 # [NM, 128, NG]
        for j in range(NG):
            p = i * NG + j
            b = p % B
            cosg[p] = cg[b // 128, b % 128, j]

    cosg = cosg.reshape(4, B)
    # device summed exp(f32(S * cos)); mirror the scaling rounding
    slog = (np.float32(S) * cosg).astype(np.float32).astype(np.float64)
    cosg64 = cosg.astype(np.float64)

    lse = np.empty(B, dtype=np.float64)
    tgt_logit = np.empty((4, B), dtype=np.float64)
    for b in range(B):
        mods: dict[int, float] = {}
        mods[int(tgts[0][b])] = S * (cosg64[0, b] - MARGIN)
        mods[int(tgts[1][b])] = cosg64[1, b] - MARGIN
        mods[int(tgts[2][b])] = cosg64[2, b] - MARGIN
        mods[int(tgts[3][b])] = cosg64[3, b] - MARGIN
        delta = 0.0
        seen = set()
        for k in range(4):
            c = int(tgts[k][b])
            if c not in seen:
                seen.add(c)
                delta += np.exp(mods[c]) - np.exp(slog[k, b])
        lse[b] = np.log(sumexp[b] + delta)
        for k in range(4):
            tgt_logit[k, b] = mods[int(tgts[k][b])]

    coeff = np.array([lam * 0.2, lam * 0.8, (1.0 - lam) * 0.2, (1.0 - lam) * 0.8])
    loss = lse.mean() - (coeff[:, None] * tgt_logit).sum(0).mean()
    return np.asarray(loss, dtype=np.float32)


# revision 26
# speedup vs baseline: 1.0093x; 1.0093x over previous
"""AM-softmax mixup loss (nn_MixupTrainLoss) on 8 TRN2 NeuronCores.

Strategy (classification / tensor parallel over the class dim, per the
sharding hint):
  - Host: L2-normalize x [512,256] and W [100000,256] rows, transpose W,
    cast to fp16 (measured end-to-end loss error ~1e-6).
  - Shard W.T column-wise: core i gets classes [12500*i, 12500*(i+1)).
    Slab layout per core: [256 gathered target cols | 12500 real | 44 pad].
  - Each core: cos = x_norm @ wn.T slab via fp16 matmuls (PE, fp32 PSUM),
    exp(S*cos) + row-sum fused on ScalarE over 2048-wide PSUM groups
    (accum_out) -> local sum_c exp(S*cos).
  - The <=4 margin-modified entries per row only perturb the softmax sum
    at <=4 columns; the gathered W columns reproduce the slab cos values
    bit-identically, so the host applies the exact margin/overwrite
    corrections analytically, merges the 8 partial sums, and finishes the
    (tiny) cross-entropy reduction in float64.
"""
import os

import numpy as np

import concourse.bacc as bacc
import concourse.bass as bass
import concourse.tile as tile
from concourse import mybir
from concourse.bass_utils import run_bass_kernel_spmd

F32 = mybir.dt.float32
F16 = mybir.dt.float16

B = 512          # batch
D = 256          # feature dim
C = 100000       # num classes
S = 30.0         # AM-softmax scale
MARGIN = 0.2     # AM-softmax margin
EPS = 1e-12
NCORES = 8
CLOC = C // NCORES          # 12500 real classes per core
CHUNK = 512                 # matmul moving free dim (one fp32 PSUM bank)
NCHUNK = 25                 # 25 chunks of 512 = 12800 slab cols per core
NG = 4 * B // NCORES        # 256 gathered target cols (slab cols [0, 256))
REAL_END = NG + CLOC        # 12756; slab cols [12756, 12800) are zero pad
NM = B // 128               # 4 row tiles of 128
GSIZES = [1, 2, 4, 4, 4, 4, 4, 2]  # chunk groups (4 banks max), small first for fast pipeline fill

_CACHE: dict = {}


def _build():
    if "nc" in _CACHE:
        return _CACHE["nc"]
    nc = bacc.Bacc("TRN2", target_bir_lowering=False, debug=False)
    wT = nc.dram_tensor("wT", [D, NCHUNK * CHUNK], F16, kind="ExternalInput")
    xT = nc.dram_tensor("xT", [D, B], F16, kind="ExternalInput")
    sumexp = nc.dram_tensor("sumexp", [128, NM * 16], F32, kind="ExternalOutput")
    cosg = nc.dram_tensor("cosg", [NM, 128, NG], F32, kind="ExternalOutput")

    def wT_g(c0, w):
        return wT[:, c0:c0 + w].rearrange("(kh p) n -> p kh n", p=128)

    xT_t = xT.rearrange("(kh p) b -> p kh b", p=128)

    with tile.TileContext(nc) as tc:
        with (
            tc.tile_pool(name="xpool", bufs=1) as xpool,
            tc.tile_pool(name="wpool", bufs=1) as wpool,
            tc.tile_pool(name="epool", bufs=2) as epool,
            tc.tile_pool(name="apool", bufs=1) as apool,
            tc.tile_pool(name="opool", bufs=1) as opool,
            tc.tile_pool(name="gpool", bufs=4) as gpool,
            tc.tile_pool(name="ps", bufs=2, space="PSUM") as pspool,
        ):
            t_x = xpool.tile([128, 2, B], F16)
            nc.sync.dma_start(t_x[:], xT_t)

            wgs = []
            ch0 = 0
            for g, gs in enumerate(GSIZES):
                t_w = wpool.tile([128, 2, gs * CHUNK], F16, tag=f"w{g}", name=f"w{g}")
                nc.sync.dma_start(
                    t_w[:], wT_g(ch0 * CHUNK, gs * CHUNK))
                wgs.append(t_w)
                ch0 += gs

            acc_all = apool.tile([128, NM * 16], F32, name="acc_all")
            nc.gpsimd.memset(acc_all[:], 0.0)

            # tiny warm-up exp so the ACT table load happens during the
            # initial DMA wait instead of on the critical path
            t_wu = opool.tile([128, 1], F32, name="warmup")
            nc.gpsimd.memset(t_wu[:], 0.0)
            nc.scalar.activation(
                t_wu[:], t_wu[:], mybir.ActivationFunctionType.Exp,
            )
            # dummy matmuls on zeros during the initial DMA wait: keeps the
            # PE busy >3.4us so the HAM clock gate opens before real work
            t_z = opool.tile([128, CHUNK], F16, name="warmz")
            nc.vector.memset(t_z[:], 0.0)
            ps_wu = pspool.tile([128, 4 * CHUNK], F32, tag="ps")
            for r in range(8):
                nc.tensor.matmul(
                    ps_wu[:, 0:CHUNK], t_z[:, 0:128], t_z[:],
                    start=True, stop=True,
                )

            ch0 = 0
            for g, gs in enumerate(GSIZES):
                W = gs * CHUNK
                for m in range(NM):
                    ps = pspool.tile([128, 4 * CHUNK], F32, tag="ps")
                    for j in range(gs):
                        for kh in range(2):
                            nc.tensor.matmul(
                                ps[:, j * CHUNK:(j + 1) * CHUNK],
                                t_x[:, kh, bass.ts(m, 128)],
                                wgs[g][:, kh, j * CHUNK:(j + 1) * CHUNK],
                                start=(kh == 0),
                                stop=(kh == 1),
                            )
                    # exp over the real-class region of this group
                    lo = NG if g == 0 else 0
                    hi = min(W, REAL_END - ch0 * CHUNK)
                    t_exp = epool.tile([128, 4 * CHUNK], F32, tag="exp")
                    nc.scalar.activation(
                        t_exp[:, lo:hi],
                        ps[:, lo:hi],
                        mybir.ActivationFunctionType.Exp,
                        scale=S,
                        accum_out=acc_all[:, m * 16 + g:m * 16 + g + 1],
                    )
                    if g == 0:
                        t_g = gpool.tile([128, NG], F32, tag="g")
                        nc.vector.tensor_copy(t_g[:], ps[:, 0:NG])
                        nc.gpsimd.dma_start(cosg[m], t_g[:])
                ch0 += gs

            nc.sync.dma_start(sumexp[:], acc_all[:])

    nc.finalize()
    _CACHE["nc"] = nc
    return nc


def kernel(inputs, weight, lam, targets1, pre1, targets2, pre2):
    inputs = np.asarray(inputs, dtype=np.float32)
    weight = np.asarray(weight, dtype=np.float32)
    lam = float(np.asarray(lam))
    tgts = [np.asarray(t).astype(np.int64) for t in (targets1, pre1, targets2, pre2)]

    # ---- host prep: normalize (float64 for accuracy), cast to fp16 ----
    x = inputs[:, :, 0].astype(np.float64)
    xn = (x / np.maximum(np.sqrt((x * x).sum(1, keepdims=True)), EPS)).astype(np.float16)
    w = weight.astype(np.float64)
    wn = (w / np.maximum(np.sqrt((w * w).sum(1, keepdims=True)), EPS)).astype(np.float16)

    xT = np.ascontiguousarray(xn.T)                      # [256, 512] fp16

    # pair p = k*B + b  (k in 0..3 over [targets1, pre1, targets2, pre2])
    cols = np.concatenate([t for t in tgts])             # [2048]

    in_maps = []
    for i in range(NCORES):
        slab = np.zeros((D, NCHUNK * CHUNK), dtype=np.float16)
        slab[:, :NG] = wn[cols[i * NG:(i + 1) * NG]].T   # gathered target cols
        slab[:, NG:REAL_END] = wn[i * CLOC:(i + 1) * CLOC].T
        in_maps.append({"wT": slab, "xT": xT})

    nc = _build()
    trace = bool(int(os.environ.get("KERNEL_TRACE", "0")))
    res = run_bass_kernel_spmd(nc, in_maps, core_ids=list(range(NCORES)), trace=trace)
    kernel.last_results = res

    # ---- host combine (float64, tiny) ----
    sumexp = np.zeros(B, dtype=np.float64)
    cosg = np.empty(4 * B, dtype=np.float32)             # raw cos at pair cols
    for i, out in enumerate(res.results):
        se = out["sumexp"].astype(np.float64)            # [128, NM*16]
        se = se.reshape(128, NM, 16).sum(2)              # [128, NM]
        sumexp += se.T.reshape(B)                        # b = m*128 + p
        cg = out["cosg"]                                 # [NM, 128, NG]
        for j in range(NG):
            p = i * NG + j
            b = p % B
            cosg[p] = cg[b // 128, b % 128, j]

    cosg = cosg.reshape(4, B)
    # device summed exp(f32(S * cos)); mirror the scaling rounding
    slog = (np.float32(S) * cosg).astype(np.float32).astype(np.float64)
    cosg64 = cosg.astype(np.float64)

    lse = np.empty(B, dtype=np.float64)
    tgt_logit = np.empty((4, B), dtype=np.float64)
    for b in range(B):
        mods: dict[int, float] = {}
        mods[int(tgts[0][b])] = S * (cosg64[0, b] - MARGIN)
        mods[int(tgts[1][b])] = cosg64[1, b] - MARGIN
        mods[int(tgts[2][b])] = cosg64[2, b] - MARGIN
        mods[int(tgts[3][b])] = cosg64[3, b] - MARGIN
        delta = 0.0
        seen = set()
        for k in range(4):
            c = int(tgts[k][b])
            if c not in seen:
                seen.add(c)
                delta += np.exp(mods[c]) - np.exp(slog[k, b])
        lse[b] = np.log(sumexp[b] + delta)
        for k in range(4):
            tgt_logit[k, b] = mods[int(tgts[k][b])]

    coeff = np.array([lam * 0.2, lam * 0.8, (1.0 - lam) * 0.2, (1.0 - lam) * 0.8])
    loss = lse.mean() - (coeff[:, None] * tgt_logit).sum(0).mean()
    return np.asarray(loss, dtype=np.float32)
